# revision 59
# baseline (speedup 1.0000x reference)
"""Single-head causal attention (B=4, T=4096, E=1024, H=128) on 8 trn2 cores.

Sharding: core c -> (batch b = c//2, piece p = c%2). Within a batch the 32
query blocks of 128 rows are split even/odd between the two pieces so the
causal workload balances. The device program is identical on all cores
(SPMD); all per-core differences are carried by the input data (gathered
query rows + a per-strip-block 0/1 mask tile).

Device algorithm (per core, all "transposed" layouts):
  QT = Wq @ xq^T           [H=128, 2048]   (gathered query rows)
  KT = Wk @ x^T            [H=128, 4096]
  VT = Wv @ x^T  -> PE transpose -> V blocks [128 tok, 128 h]
  per q-tile (512 queries = in-tile blocks i=0..3):
    for kb in key blocks (causal bound c0 = 128*floor(j/2), j = kb-8tt):
      ST[kb] = KT_blk^T @ QT_tile   [128 k, 512-c0 q] (PSUM)
      PT = exp(scale * ST)          (ACT, PSUM->SBUF f16)
      strip blocks: PT[:, c0:c0+128] *= mask8[j]   (DVE f16; mask is
        tri / zeros / ones depending on parity -- per-core data)
      acc (+)= PT                   (DVE f16 accumulate; replaces the
                                     ones^T @ PT matmuls of the baseline)
      OT += V_blk^T @ PT            [128 h, 512 q] (PSUM accum)
    per i-block: lc = acc_blk^T @ ones -> 1/l (DVE reciprocal)
    O^T -> PE transpose -> per-partition scalar multiply -> store

Scheduling: the attention inner loop is paced by the scalar engine's exp
(529 ns/block vs the PE's 430 ns), so the NEXT round's projection matmuls
are fed to the PE in small chunks between attention steps (the "stream");
the PE then never idles waiting on exp. Round-0 x loads are split in half
so the first projections start as soon as half the data has landed.
"""

import numpy as np

B, T, E, H = 4, 4096, 1024, 128
P = 128
NB_E = E // P           # 8 contraction chunks
TQ = T // 2             # 2048 gathered queries per core
N_QT = TQ // 512        # 4 q-tiles per core
SCALE = float(H) ** -0.5
N_CORES = 8
F32 = np.float32
N_WARMUP = 0


def _query_rows(p: int) -> np.ndarray:
    """Absolute row indices of the gathered queries for piece p (in order)."""
    blocks = [np.arange(256 * g + 128 * p, 256 * g + 128 * p + 128) for g in range(16)]
    return np.concatenate(blocks)


def _mask8(p: int) -> np.ndarray:
    """mask8[kk, j, r]: multiplier for strip block j at query block i=c0/128.

    Query col 128*i + r is abs 1024*tt + 256*i + 128*p + r; key kk of block
    kb is abs 128*kb + kk. With j = kb - 8*tt, m = 2*i + p - j the block is
    fully visible (m>=1), triangular (m==0, visible iff kk<=r), or hidden
    (m<=-1). At i = floor(j/2): m = 2*floor(j/2) + p - j.
    """
    kk = np.arange(P)[:, None]
    r = np.arange(P)[None, :]
    tri = (kk <= r)
    m8 = np.empty((P, 8, P), dtype=np.float16)
    for j in range(8):
        m = 2 * (j // 2) + p - j
        if m >= 1:
            m8[:, j, :] = 1.0
        elif m == 0:
            m8[:, j, :] = tri.astype(np.float16)
        else:
            m8[:, j, :] = 0.0
    return m8


def _emit(tc, aps):
    from concourse import mybir
    from concourse.masks import make_identity

    nc = tc.nc
    f32 = mybir.dt.float32
    f16 = mybir.dt.float16
    EXP = mybir.ActivationFunctionType.Exp

    xT, xqT, wq, wk, wv, mask8, out = aps

    from contextlib import ExitStack

    ctx = ExitStack()
    with ctx:
        # ---- pools ----
        consts = ctx.enter_context(tc.tile_pool(name="consts", bufs=1))
        x_pool = ctx.enter_context(tc.tile_pool(name="x", bufs=14))
        vt_pool = ctx.enter_context(tc.tile_pool(name="vt", bufs=3))
        pt_pool = ctx.enter_context(tc.tile_pool(name="pt", bufs=8))
        acc_pool = ctx.enter_context(tc.tile_pool(name="acc", bufs=4))
        osb_pool = ctx.enter_context(tc.tile_pool(name="osb", bufs=2))
        on_pool = ctx.enter_context(tc.tile_pool(name="on", bufs=2))
        sm_pool = ctx.enter_context(tc.tile_pool(name="sm", bufs=8))
        s_ps = ctx.enter_context(tc.tile_pool(name="sps", bufs=3, space="PSUM"))
        proj_ps = ctx.enter_context(tc.tile_pool(name="pps", bufs=1, space="PSUM"))
        o_ps = ctx.enter_context(tc.tile_pool(name="ops", bufs=2, space="PSUM"))
        t_ps = ctx.enter_context(tc.tile_pool(name="tps", bufs=2, space="PSUM"))

        # ---- persistent SBUF tensors ----
        identity = consts.tile([P, P], f16)
        ones = consts.tile([P, 1], f16)
        wq_sb = consts.tile([P, NB_E, P], f16)
        wk_sb = consts.tile([P, NB_E, P], f16)
        wv_sb = consts.tile([P, NB_E, P], f16)
        mask_sb = consts.tile([P, 8, P], f16)
        kt_all = consts.tile([P, T], f16)
        v_all = consts.tile([P, T // P, P], f16)
        qt_all = consts.tile([P, TQ], f16)

        make_identity(nc, identity[:])
        nc.gpsimd.memset(ones[:], 1.0)

        # PE warmup: ramp the tensor-engine p-state during the initial DMA
        # window (transposes depend only on the on-device identity tile).
        for w in range(N_WARMUP):
            wp = t_ps.tile([P, P], f16, tag="tps", name=f"warm_{w}")
            nc.tensor.transpose(wp[:], identity[:], identity[:])

        def load_x(src_ap, t0):
            xt = x_pool.tile([P, NB_E, 512], f16, tag="x")
            nc.sync.dma_start(
                xt[:], src_ap[:, t0:t0 + 512].rearrange("(c p) t -> p c t", p=P))
            return xt

        def load_x_split(src_ap, t0):
            # two half-chunk DMAs in separate tiles: the first projection
            # matmuls can start after only half the data has landed
            halves = []
            for h in range(2):
                xt = x_pool.tile([P, NB_E // 2, 512], f16, tag="x")
                nc.sync.dma_start(
                    xt[:],
                    src_ap[h * 512:(h + 1) * 512, t0:t0 + 512]
                    .rearrange("(c p) t -> p c t", p=P))
                halves.append(xt)
            return halves

        # interleave weight loads with round-0 x loads so each projection's
        # dependencies arrive as early as possible
        nc.sync.dma_start(wq_sb[:], wq.rearrange("(c p) h -> p c h", p=P))
        xq_pre = load_x_split(xqT, 0)
        nc.sync.dma_start(wk_sb[:], wk.rearrange("(c p) h -> p c h", p=P))
        xk_pre = [load_x_split(xT, 0)]
        nc.sync.dma_start(wv_sb[:], wv.rearrange("(c p) h -> p c h", p=P))
        nc.sync.dma_start(mask_sb[:], mask8.rearrange("(j p) r -> p j r", p=P))
        xk_pre.append(load_x_split(xT, 512))

        def x_src(xt, c):
            return xt[c // 4][:, c % 4, :] if isinstance(xt, list) else xt[:, c, :]

        def project(w_sb, xt, dst_ap):
            ps = proj_ps.tile([P, 512], f32, tag="pps")
            for c in range(NB_E):
                nc.tensor.matmul(
                    ps[:],
                    lhsT=w_sb[:, c, :],
                    rhs=x_src(xt, c),
                    start=(c == 0),
                    stop=(c == NB_E - 1),
                )
            nc.vector.tensor_copy(dst_ap, ps[:])

        def v_transpose(vt, tok, u):
            kb = tok * 4 + u
            tp = t_ps.tile([P, P], f16, tag="tps", name=f"vt_{kb}")
            nc.tensor.transpose(tp[:], vt[:, u * P:(u + 1) * P], identity[:])
            nc.vector.tensor_copy(v_all[:, kb, :], tp[:])

        def do_kv(tok):
            xk = xk_pre[tok] if tok < 2 else load_x(xT, tok * 512)
            project(wk_sb, xk, kt_all[:, tok * 512:(tok + 1) * 512])
            vt = vt_pool.tile([P, 512], f16, tag="vt")
            project(wv_sb, xk, vt[:])
            for u in range(4):
                v_transpose(vt, tok, u)

        def do_q(tt):
            xq = xq_pre if tt == 0 else load_x(xqT, tt * 512)
            project(wq_sb, xq, qt_all[:, tt * 512:(tt + 1) * 512])

        # ---- projection step-stream: next-round projection work emitted in
        # small chunks between attention steps, so the tensor engine has
        # filler while it waits on exp results (the scalar engine paces the
        # attention inner loop otherwise) ----
        stream = []
        _psn = [0]

        def stream_project(w_sb, xt, dst_ap):
            ref = {}

            def mk(c):
                def f():
                    if c == 0:
                        _psn[0] += 1
                        ref['ps'] = proj_ps.tile(
                            [P, 512], f32, tag="pps", name=f"pps_{_psn[0]}")
                    nc.tensor.matmul(
                        ref['ps'][:],
                        lhsT=w_sb[:, c, :],
                        rhs=x_src(xt, c),
                        start=(c == 0),
                        stop=(c == NB_E - 1),
                    )
                return f

            for c in range(NB_E):
                stream.append(mk(c))
            stream.append(lambda: nc.vector.tensor_copy(dst_ap, ref['ps'][:]))

        def stream_kv(tok):
            xk = xk_pre[tok] if tok < 2 else load_x(xT, tok * 512)
            stream_project(wk_sb, xk, kt_all[:, tok * 512:(tok + 1) * 512])
            vt = vt_pool.tile([P, 512], f16, tag="vt")
            stream_project(wv_sb, xk, vt[:])
            for u in range(4):
                stream.append(
                    (lambda uu: lambda: v_transpose(vt, tok, uu))(u))

        def stream_q(tt):
            xq = load_x(xqT, tt * 512)
            stream_project(wq_sb, xq, qt_all[:, tt * 512:(tt + 1) * 512])

        def emit_stream(n):
            for _ in range(min(n, len(stream))):
                stream.pop(0)()

        for tt in range(N_QT):
            if tt == 0:
                do_q(0)
                do_kv(0)
            stream_kv(2 * tt + 1)

            # ---- attention for q-tile tt ----
            qs = qt_all[:, tt * 512:(tt + 1) * 512]
            ot = o_ps.tile([P, 512], f32, tag="ops")
            acc = acc_pool.tile([P, 512], f16, tag="acc")
            o_sb = osb_pool.tile([P, 512], f16, tag="osb")
            on = on_pool.tile([P, 4, P], f32, tag="on")
            nkb = 8 * tt + 8

            def c0_of(kb):
                j = kb - 8 * tt
                return 128 * (j // 2) if j > 0 else 0

            s_tiles = {}

            def emit_scores(kb):
                c0 = c0_of(kb)
                s = s_ps.tile([P, 512], f32, tag="sps", name=f"s_{tt}_{kb}")
                nc.tensor.matmul(
                    s[:, c0:512],
                    lhsT=kt_all[:, kb * P:(kb + 1) * P],
                    rhs=qs[:, c0:512],
                    start=True,
                    stop=True,
                )
                s_tiles[kb] = s

            rlcs = [None] * 4

            def lc_block(i):
                # acc columns for block i are final once every kb with
                # c0 <= 128*i has run (kb = 8tt+2i+1); compute 1/l for the
                # block early so only the PSUM reads remain for the tail.
                lc = t_ps.tile([P, 1], f32, tag="tps", name=f"lc_{tt}_{i}")
                nc.tensor.matmul(
                    lc[:],
                    lhsT=acc[:, i * P:(i + 1) * P],
                    rhs=ones[:],
                    start=True,
                    stop=True,
                )
                rlc = sm_pool.tile([P, 1], f32, tag="rlc")
                nc.vector.reciprocal(rlc[:], lc[:])
                rlcs[i] = rlc

            pending_adds = []

            def mk_add(kb, c0, pt):
                def f():
                    if kb == 0:
                        nc.vector.tensor_copy(acc[:, c0:512], pt[:, c0:512])
                    else:
                        nc.vector.tensor_add(
                            acc[:, c0:512], acc[:, c0:512], pt[:, c0:512])
                return f

            def attn_half(kbs):
                pace = -(-len(stream) // len(kbs)) if kbs else 0
                for idx, kb in enumerate(kbs):
                    if idx == 0:
                        emit_scores(kb)
                        if len(kbs) > 1:
                            emit_scores(kbs[1])
                    if idx + 2 < len(kbs):
                        emit_scores(kbs[idx + 2])
                    s = s_tiles.pop(kb)
                    c0 = c0_of(kb)
                    j = kb - 8 * tt
                    pt = pt_pool.tile([P, 512], f16, tag="pt")
                    nc.scalar.activation(
                        pt[:, c0:512], s[:, c0:512], EXP, scale=SCALE)
                    if j >= 0:
                        nc.vector.tensor_mul(
                            pt[:, c0:c0 + P], pt[:, c0:c0 + P], mask_sb[:, j, :])
                    # defer the l-accumulate by 2 blocks so the mask multiply
                    # (which gates PV) isn't queued behind the add backlog
                    pending_adds.append(mk_add(kb, c0, pt))
                    if len(pending_adds) > 2:
                        pending_adds.pop(0)()
                    nc.tensor.matmul(
                        ot[:, c0:512],
                        lhsT=v_all[:, kb, :],
                        rhs=pt[:, c0:512],
                        start=(kb == 0),
                        stop=(kb == nkb - 1),
                    )
                    emit_stream(pace)

            attn_half(list(range(8 * tt + 4)))
            emit_stream(len(stream))    # kv(2tt+1) must be done for half B
            if tt + 1 < N_QT:
                stream_q(tt + 1)
                stream_kv(2 * tt + 2)
            attn_half(list(range(8 * tt + 4, nkb)))
            for f in pending_adds:      # acc must be complete before lc
                f()
            pending_adds.clear()
            emit_stream(len(stream))    # flush before round tt+1

            # ---- epilogue: normalize + transpose + store per column block ----
            for i in range(4):
                lc_block(i)
            last = tt == N_QT - 1
            nc.vector.tensor_copy(o_sb[:], ot[:])
            for i in range(4):
                tp = t_ps.tile([P, P], f16, tag="tps", name=f"otp_{tt}_{i}")
                nc.tensor.transpose(tp[:], o_sb[:, i * P:(i + 1) * P], identity[:])
                nc.vector.tensor_scalar_mul(on[:, i, :], tp[:], rlcs[i][:])
                if not last:
                    nc.sync.dma_start(
                        out[tt * 512 + i * P:tt * 512 + (i + 1) * P, :],
                        on[:, i, :])
            if last:
                # single store for the final tile: one queue dispatch instead
                # of four on the tail critical path
                nc.sync.dma_start(
                    out[tt * 512:(tt + 1) * 512, :].rearrange(
                        "(i p) h -> p i h", p=P),
                    on[:],
                )


def build_program():
    import concourse.tile as tile
    from concourse import bacc, mybir

    f32 = mybir.dt.float32
    f16 = mybir.dt.float16
    nc = bacc.Bacc("TRN2", target_bir_lowering=False, debug=False,
                   num_devices=N_CORES)
    xT = nc.dram_tensor("xT", [E, T], f16, kind="ExternalInput").ap()
    xqT = nc.dram_tensor("xqT", [E, TQ], f16, kind="ExternalInput").ap()
    wq = nc.dram_tensor("wq", [E, H], f16, kind="ExternalInput").ap()
    wk = nc.dram_tensor("wk", [E, H], f16, kind="ExternalInput").ap()
    wv = nc.dram_tensor("wv", [E, H], f16, kind="ExternalInput").ap()
    mask8 = nc.dram_tensor("mask8", [8 * P, P], f16, kind="ExternalInput").ap()
    out = nc.dram_tensor("out", [TQ, H], f32, kind="ExternalOutput").ap()

    with tile.TileContext(nc) as tc:
        _emit(tc, (xT, xqT, wq, wk, wv, mask8, out))
    nc.compile()
    return nc


def make_in_maps(x, Wq, Wk, Wv):
    """Per-core input maps. x: [B,T,E] f32; W*: [H,E] f32."""
    x = np.asarray(x, dtype=F32)
    wq_t = np.ascontiguousarray(np.asarray(Wq, dtype=F32).T.astype(np.float16))
    wk_t = np.ascontiguousarray(np.asarray(Wk, dtype=F32).T.astype(np.float16))
    wv_t = np.ascontiguousarray(np.asarray(Wv, dtype=F32).T.astype(np.float16))
    in_maps = []
    for c in range(N_CORES):
        b, p = c // 2, c % 2
        xb = x[b]                                              # [T, E]
        xT_np = np.ascontiguousarray(xb.T.astype(np.float16))
        xqT_np = np.ascontiguousarray(xb[_query_rows(p)].T.astype(np.float16))
        in_maps.append({
            "xT": xT_np,
            "xqT": xqT_np,
            "wq": wq_t,
            "wk": wk_t,
            "wv": wv_t,
            "mask8": _mask8(p).transpose(1, 0, 2).reshape(8 * P, P),
        })
    return in_maps


LDW_OPT = False


def _enable_ldw_opt():
    """Flip walrus's --enable-ldw-opt (defaults off in this toolchain): it
    elides redundant LDWEIGHTS when consecutive matmuls share a stationary
    operand. Correctness is covered by the output check."""
    import concourse.bass_utils as bu
    if getattr(bu, "_ldw_patched", False):
        return
    orig = bu.run_command

    def patched(cmd, *a, **kw):
        cmd = ["--enable-ldw-opt=true" if c == "--enable-ldw-opt=false" else c
               for c in cmd]
        return orig(cmd, *a, **kw)

    bu.run_command = patched
    bu._ldw_patched = True


def run(x, Wq, Wk, Wv, trace=False, trace_cores=None):
    """Returns (full_output [B,T,H] f32, BassKernelResults)."""
    from concourse.bass_utils import run_bass_kernel_spmd

    if LDW_OPT:
        _enable_ldw_opt()
    nc = build_program()
    in_maps = make_in_maps(x, Wq, Wk, Wv)
    res = run_bass_kernel_spmd(
        nc, in_maps, list(range(N_CORES)), trace=trace,
        trace_cores=trace_cores,
    )
    full = np.empty((B, T, H), dtype=F32)
    for c in range(N_CORES):
        b, p = c // 2, c % 2
        full[b, _query_rows(p), :] = res.results[c]["out"]
    return full, res


def kernel(x, Wq, Wk, Wv):
    full, _ = run(x, Wq, Wk, Wv, trace=False)
    return full


if __name__ == "__main__":
    nc = build_program()
    print("program built ok")


# revision 60
# speedup vs baseline: 1.0703x; 1.0703x over previous
"""Single-head causal attention (B=4, T=4096, E=1024, H=128) on 8 trn2 cores.

Sharding: core c -> (batch b = c//2, piece p = c%2). Within a batch the 32
query blocks of 128 rows are split even/odd between the two pieces so the
causal workload balances. The device program is identical on all cores
(SPMD); all per-core differences are carried by the input data (gathered
query rows + a per-strip-block 0/1 mask tile).

Device algorithm (per core, all "transposed" layouts):
  QT = Wq @ xq^T           [H=128, 2048]   (gathered query rows)
  KT = Wk @ x^T            [H=128, 4096]
  VT = Wv @ x^T  -> PE transpose -> V blocks [128 tok, 128 h]
  per q-tile (512 queries = in-tile blocks i=0..3):
    for kb in key blocks (causal bound c0 = 128*floor(j/2), j = kb-8tt):
      ST[kb] = KT_blk^T @ QT_tile   [128 k, 512-c0 q] (PSUM)
      PT = exp(scale * ST)          (ACT, PSUM->SBUF f16)
      strip blocks: PT[:, c0:c0+128] *= mask8[j]   (DVE f16; mask is
        tri / zeros / ones depending on parity -- per-core data)
      acc (+)= PT                   (DVE f16 accumulate; replaces the
                                     ones^T @ PT matmuls of the baseline)
      OT += V_blk^T @ PT            [128 h, 512 q] (PSUM accum)
    per i-block: lc = acc_blk^T @ ones -> 1/l (DVE reciprocal)
    O^T -> PE transpose -> per-partition scalar multiply -> store

Scheduling: the attention inner loop is paced by the scalar engine's exp
(529 ns/block vs the PE's 430 ns), so the NEXT round's projection matmuls
are fed to the PE in small chunks between attention steps (the "stream");
the PE then never idles waiting on exp. Round-0 x loads are split in half
so the first projections start as soon as half the data has landed.
"""

import numpy as np

B, T, E, H = 4, 4096, 1024, 128
P = 128
NB_E = E // P           # 8 contraction chunks
TQ = T // 2             # 2048 gathered queries per core
N_QT = TQ // 512        # 4 q-tiles per core
SCALE = float(H) ** -0.5
N_CORES = 8
F32 = np.float32
N_WARMUP = 0


def _query_rows(p: int) -> np.ndarray:
    """Absolute row indices of the gathered queries for piece p (in order)."""
    blocks = [np.arange(256 * g + 128 * p, 256 * g + 128 * p + 128) for g in range(16)]
    return np.concatenate(blocks)


def _mask8(p: int) -> np.ndarray:
    """mask8[kk, j, r]: multiplier for strip block j at query block i=c0/128.

    Query col 128*i + r is abs 1024*tt + 256*i + 128*p + r; key kk of block
    kb is abs 128*kb + kk. With j = kb - 8*tt, m = 2*i + p - j the block is
    fully visible (m>=1), triangular (m==0, visible iff kk<=r), or hidden
    (m<=-1). At i = floor(j/2): m = 2*floor(j/2) + p - j.
    """
    kk = np.arange(P)[:, None]
    r = np.arange(P)[None, :]
    tri = (kk <= r)
    m8 = np.empty((P, 8, P), dtype=np.float16)
    for j in range(8):
        m = 2 * (j // 2) + p - j
        if m >= 1:
            m8[:, j, :] = 1.0
        elif m == 0:
            m8[:, j, :] = tri.astype(np.float16)
        else:
            m8[:, j, :] = 0.0
    return m8


def _emit(tc, aps):
    from concourse import mybir
    from concourse.masks import make_identity

    nc = tc.nc
    f32 = mybir.dt.float32
    f16 = mybir.dt.float16
    EXP = mybir.ActivationFunctionType.Exp

    xT, xqT, wq, wk, wv, mask8, out = aps

    from contextlib import ExitStack

    ctx = ExitStack()
    with ctx:
        # ---- pools ----
        consts = ctx.enter_context(tc.tile_pool(name="consts", bufs=1))
        x_pool = ctx.enter_context(tc.tile_pool(name="x", bufs=14))
        vt_pool = ctx.enter_context(tc.tile_pool(name="vt", bufs=3))
        pt_pool = ctx.enter_context(tc.tile_pool(name="pt", bufs=8))
        acc_pool = ctx.enter_context(tc.tile_pool(name="acc", bufs=4))
        osb_pool = ctx.enter_context(tc.tile_pool(name="osb", bufs=2))
        on_pool = ctx.enter_context(tc.tile_pool(name="on", bufs=2))
        sm_pool = ctx.enter_context(tc.tile_pool(name="sm", bufs=8))
        s_ps = ctx.enter_context(tc.tile_pool(name="sps", bufs=3, space="PSUM"))
        proj_ps = ctx.enter_context(tc.tile_pool(name="pps", bufs=2, space="PSUM"))
        o_ps = ctx.enter_context(tc.tile_pool(name="ops", bufs=1, space="PSUM"))
        t_ps = ctx.enter_context(tc.tile_pool(name="tps", bufs=2, space="PSUM"))

        # ---- persistent SBUF tensors ----
        identity = consts.tile([P, P], f16)
        ones = consts.tile([P, 1], f16)
        wq_sb = consts.tile([P, NB_E, P], f16)
        wk_sb = consts.tile([P, NB_E, P], f16)
        wv_sb = consts.tile([P, NB_E, P], f16)
        mask_sb = consts.tile([P, 8, P], f16)
        kt_all = consts.tile([P, T], f16)
        v_all = consts.tile([P, T // P, P], f16)
        qt_all = consts.tile([P, TQ], f16)

        make_identity(nc, identity[:])
        nc.gpsimd.memset(ones[:], 1.0)

        # PE warmup: ramp the tensor-engine p-state during the initial DMA
        # window (transposes depend only on the on-device identity tile).
        for w in range(N_WARMUP):
            wp = t_ps.tile([P, P], f16, tag="tps", name=f"warm_{w}")
            nc.tensor.transpose(wp[:], identity[:], identity[:])

        def load_x(src_ap, t0):
            xt = x_pool.tile([P, NB_E, 512], f16, tag="x")
            nc.sync.dma_start(
                xt[:], src_ap[:, t0:t0 + 512].rearrange("(c p) t -> p c t", p=P))
            return xt

        def load_x_split(src_ap, t0):
            # two half-chunk DMAs in separate tiles: the first projection
            # matmuls can start after only half the data has landed
            halves = []
            for h in range(2):
                xt = x_pool.tile([P, NB_E // 2, 512], f16, tag="x")
                nc.sync.dma_start(
                    xt[:],
                    src_ap[h * 512:(h + 1) * 512, t0:t0 + 512]
                    .rearrange("(c p) t -> p c t", p=P))
                halves.append(xt)
            return halves

        # interleave weight loads with round-0 x loads so each projection's
        # dependencies arrive as early as possible
        nc.sync.dma_start(wq_sb[:], wq.rearrange("(c p) h -> p c h", p=P))
        xq_pre = load_x_split(xqT, 0)
        nc.sync.dma_start(wk_sb[:], wk.rearrange("(c p) h -> p c h", p=P))
        xk_pre = [load_x_split(xT, 0)]
        nc.sync.dma_start(wv_sb[:], wv.rearrange("(c p) h -> p c h", p=P))
        nc.sync.dma_start(mask_sb[:], mask8.rearrange("(j p) r -> p j r", p=P))
        xk_pre.append(load_x_split(xT, 512))

        def x_src(xt, c):
            return xt[c // 4][:, c % 4, :] if isinstance(xt, list) else xt[:, c, :]

        def project(w_sb, xt, dst_ap):
            ps = proj_ps.tile([P, 512], f32, tag="pps")
            for c in range(NB_E):
                nc.tensor.matmul(
                    ps[:],
                    lhsT=w_sb[:, c, :],
                    rhs=x_src(xt, c),
                    start=(c == 0),
                    stop=(c == NB_E - 1),
                )
            nc.vector.tensor_copy(dst_ap, ps[:])

        def v_transpose(vt, tok, u):
            kb = tok * 4 + u
            tp = t_ps.tile([P, P], f16, tag="tps", name=f"vt_{kb}")
            nc.tensor.transpose(tp[:], vt[:, u * P:(u + 1) * P], identity[:])
            nc.vector.tensor_copy(v_all[:, kb, :], tp[:])

        def do_kv(tok):
            xk = xk_pre[tok] if tok < 2 else load_x(xT, tok * 512)
            project(wk_sb, xk, kt_all[:, tok * 512:(tok + 1) * 512])
            vt = vt_pool.tile([P, 512], f16, tag="vt")
            project(wv_sb, xk, vt[:])
            for u in range(4):
                v_transpose(vt, tok, u)

        def do_q(tt):
            xq = xq_pre if tt == 0 else load_x(xqT, tt * 512)
            project(wq_sb, xq, qt_all[:, tt * 512:(tt + 1) * 512])

        # ---- projection step-stream: next-round projection work emitted in
        # small chunks between attention steps, so the tensor engine has
        # filler while it waits on exp results (the scalar engine paces the
        # attention inner loop otherwise) ----
        stream = []
        _psn = [0]

        def stream_project(w_sb, xt, dst_ap):
            ref = {}

            def mk(c):
                def f():
                    if c == 0:
                        _psn[0] += 1
                        ref['ps'] = proj_ps.tile(
                            [P, 512], f32, tag="pps", name=f"pps_{_psn[0]}")
                    nc.tensor.matmul(
                        ref['ps'][:],
                        lhsT=w_sb[:, c, :],
                        rhs=x_src(xt, c),
                        start=(c == 0),
                        stop=(c == NB_E - 1),
                    )
                return f

            for c in range(NB_E):
                stream.append(mk(c))
            stream.append(lambda: nc.vector.tensor_copy(dst_ap, ref['ps'][:]))

        def stream_kv(tok):
            xk = xk_pre[tok] if tok < 2 else load_x(xT, tok * 512)
            stream_project(wk_sb, xk, kt_all[:, tok * 512:(tok + 1) * 512])
            vt = vt_pool.tile([P, 512], f16, tag="vt")
            stream_project(wv_sb, xk, vt[:])
            for u in range(4):
                stream.append(
                    (lambda uu: lambda: v_transpose(vt, tok, uu))(u))

        def stream_q(tt):
            xq = load_x(xqT, tt * 512)
            stream_project(wq_sb, xq, qt_all[:, tt * 512:(tt + 1) * 512])

        def emit_stream(n):
            for _ in range(min(n, len(stream))):
                stream.pop(0)()

        for tt in range(N_QT):
            if tt == 0:
                do_q(0)
                do_kv(0)
            stream_kv(2 * tt + 1)

            # ---- attention for q-tile tt ----
            qs = qt_all[:, tt * 512:(tt + 1) * 512]
            ot = o_ps.tile([P, 512], f32, tag="ops")
            acc = acc_pool.tile([P, 512], f16, tag="acc")
            o_sb = osb_pool.tile([P, 512], f16, tag="osb")
            on = on_pool.tile([P, 4, P], f32, tag="on")
            nkb = 8 * tt + 8

            def c0_of(kb):
                j = kb - 8 * tt
                return 128 * (j // 2) if j > 0 else 0

            s_tiles = {}

            def emit_scores(kb):
                c0 = c0_of(kb)
                s = s_ps.tile([P, 512], f32, tag="sps", name=f"s_{tt}_{kb}")
                nc.tensor.matmul(
                    s[:, c0:512],
                    lhsT=kt_all[:, kb * P:(kb + 1) * P],
                    rhs=qs[:, c0:512],
                    start=True,
                    stop=True,
                )
                s_tiles[kb] = s

            rlcs = [None] * 4

            def lc_block(i):
                # acc columns for block i are final once every kb with
                # c0 <= 128*i has run (kb = 8tt+2i+1); compute 1/l for the
                # block early so only the PSUM reads remain for the tail.
                lc = t_ps.tile([P, 1], f32, tag="tps", name=f"lc_{tt}_{i}")
                nc.tensor.matmul(
                    lc[:],
                    lhsT=acc[:, i * P:(i + 1) * P],
                    rhs=ones[:],
                    start=True,
                    stop=True,
                )
                rlc = sm_pool.tile([P, 1], f32, tag="rlc")
                nc.vector.reciprocal(rlc[:], lc[:])
                rlcs[i] = rlc

            pending_adds = []

            def mk_add(kb, c0, pt):
                def f():
                    if kb == 0:
                        nc.vector.tensor_copy(acc[:, c0:512], pt[:, c0:512])
                    else:
                        nc.vector.tensor_add(
                            acc[:, c0:512], acc[:, c0:512], pt[:, c0:512])
                return f

            def attn_half(kbs):
                pace = -(-len(stream) // len(kbs)) if kbs else 0
                for idx, kb in enumerate(kbs):
                    if idx == 0:
                        emit_scores(kb)
                        if len(kbs) > 1:
                            emit_scores(kbs[1])
                    if idx + 2 < len(kbs):
                        emit_scores(kbs[idx + 2])
                    s = s_tiles.pop(kb)
                    c0 = c0_of(kb)
                    j = kb - 8 * tt
                    pt = pt_pool.tile([P, 512], f16, tag="pt")
                    nc.scalar.activation(
                        pt[:, c0:512], s[:, c0:512], EXP, scale=SCALE)
                    if j >= 0:
                        nc.vector.tensor_mul(
                            pt[:, c0:c0 + P], pt[:, c0:c0 + P], mask_sb[:, j, :])
                    # defer the l-accumulate by 2 blocks so the mask multiply
                    # (which gates PV) isn't queued behind the add backlog
                    pending_adds.append(mk_add(kb, c0, pt))
                    if len(pending_adds) > 2:
                        pending_adds.pop(0)()
                    nc.tensor.matmul(
                        ot[:, c0:512],
                        lhsT=v_all[:, kb, :],
                        rhs=pt[:, c0:512],
                        start=(kb == 0),
                        stop=(kb == nkb - 1),
                    )
                    emit_stream(pace)

            attn_half(list(range(8 * tt + 4)))
            emit_stream(len(stream))    # kv(2tt+1) must be done for half B
            if tt + 1 < N_QT:
                stream_q(tt + 1)
                stream_kv(2 * tt + 2)
            attn_half(list(range(8 * tt + 4, nkb)))
            for f in pending_adds:      # acc must be complete before lc
                f()
            pending_adds.clear()
            emit_stream(len(stream))    # flush before round tt+1

            # ---- epilogue: normalize + transpose + store per column block ----
            for i in range(4):
                lc_block(i)
            last = tt == N_QT - 1
            nc.vector.tensor_copy(o_sb[:], ot[:])
            for i in range(4):
                tp = t_ps.tile([P, P], f16, tag="tps", name=f"otp_{tt}_{i}")
                nc.tensor.transpose(tp[:], o_sb[:, i * P:(i + 1) * P], identity[:])
                nc.vector.tensor_scalar_mul(on[:, i, :], tp[:], rlcs[i][:])
                if not last:
                    nc.sync.dma_start(
                        out[tt * 512 + i * P:tt * 512 + (i + 1) * P, :],
                        on[:, i, :])
            if last:
                # single store for the final tile: one queue dispatch instead
                # of four on the tail critical path
                nc.sync.dma_start(
                    out[tt * 512:(tt + 1) * 512, :].rearrange(
                        "(i p) h -> p i h", p=P),
                    on[:],
                )


def build_program():
    import concourse.tile as tile
    from concourse import bacc, mybir

    f32 = mybir.dt.float32
    f16 = mybir.dt.float16
    nc = bacc.Bacc("TRN2", target_bir_lowering=False, debug=False,
                   num_devices=N_CORES)
    xT = nc.dram_tensor("xT", [E, T], f16, kind="ExternalInput").ap()
    xqT = nc.dram_tensor("xqT", [E, TQ], f16, kind="ExternalInput").ap()
    wq = nc.dram_tensor("wq", [E, H], f16, kind="ExternalInput").ap()
    wk = nc.dram_tensor("wk", [E, H], f16, kind="ExternalInput").ap()
    wv = nc.dram_tensor("wv", [E, H], f16, kind="ExternalInput").ap()
    mask8 = nc.dram_tensor("mask8", [8 * P, P], f16, kind="ExternalInput").ap()
    out = nc.dram_tensor("out", [TQ, H], f32, kind="ExternalOutput").ap()

    with tile.TileContext(nc) as tc:
        _emit(tc, (xT, xqT, wq, wk, wv, mask8, out))
    nc.compile()
    return nc


def make_in_maps(x, Wq, Wk, Wv):
    """Per-core input maps. x: [B,T,E] f32; W*: [H,E] f32."""
    x = np.asarray(x, dtype=F32)
    wq_t = np.ascontiguousarray(np.asarray(Wq, dtype=F32).T.astype(np.float16))
    wk_t = np.ascontiguousarray(np.asarray(Wk, dtype=F32).T.astype(np.float16))
    wv_t = np.ascontiguousarray(np.asarray(Wv, dtype=F32).T.astype(np.float16))
    in_maps = []
    for c in range(N_CORES):
        b, p = c // 2, c % 2
        xb = x[b]                                              # [T, E]
        xT_np = np.ascontiguousarray(xb.T.astype(np.float16))
        xqT_np = np.ascontiguousarray(xb[_query_rows(p)].T.astype(np.float16))
        in_maps.append({
            "xT": xT_np,
            "xqT": xqT_np,
            "wq": wq_t,
            "wk": wk_t,
            "wv": wv_t,
            "mask8": _mask8(p).transpose(1, 0, 2).reshape(8 * P, P),
        })
    return in_maps


LDW_OPT = False


def _enable_ldw_opt():
    """Flip walrus's --enable-ldw-opt (defaults off in this toolchain): it
    elides redundant LDWEIGHTS when consecutive matmuls share a stationary
    operand. Correctness is covered by the output check."""
    import concourse.bass_utils as bu
    if getattr(bu, "_ldw_patched", False):
        return
    orig = bu.run_command

    def patched(cmd, *a, **kw):
        cmd = ["--enable-ldw-opt=true" if c == "--enable-ldw-opt=false" else c
               for c in cmd]
        return orig(cmd, *a, **kw)

    bu.run_command = patched
    bu._ldw_patched = True


def run(x, Wq, Wk, Wv, trace=False, trace_cores=None):
    """Returns (full_output [B,T,H] f32, BassKernelResults)."""
    from concourse.bass_utils import run_bass_kernel_spmd

    if LDW_OPT:
        _enable_ldw_opt()
    nc = build_program()
    in_maps = make_in_maps(x, Wq, Wk, Wv)
    res = run_bass_kernel_spmd(
        nc, in_maps, list(range(N_CORES)), trace=trace,
        trace_cores=trace_cores,
    )
    full = np.empty((B, T, H), dtype=F32)
    for c in range(N_CORES):
        b, p = c // 2, c % 2
        full[b, _query_rows(p), :] = res.results[c]["out"]
    return full, res


def kernel(x, Wq, Wk, Wv):
    full, _ = run(x, Wq, Wk, Wv, trace=False)
    return full


if __name__ == "__main__":
    nc = build_program()
    print("program built ok")


# revision 63
# speedup vs baseline: 1.0944x; 1.0225x over previous
"""Single-head causal attention (B=4, T=4096, E=1024, H=128) on 8 trn2 cores.

Sharding: core c -> (batch b = c//2, piece p = c%2). Within a batch the 32
query blocks of 128 rows are split even/odd between the two pieces so the
causal workload balances. The device program is identical on all cores
(SPMD); all per-core differences are carried by the input data (gathered
query rows + a per-strip-block 0/1 mask tile).

Device algorithm (per core, all "transposed" layouts):
  QT = Wq @ xq^T           [H=128, 2048]   (gathered query rows)
  KT = Wk @ x^T            [H=128, 4096]
  VT = Wv @ x^T  -> PE transpose -> V blocks [128 tok, 128 h]
  per q-tile (512 queries = in-tile blocks i=0..3):
    for kb in key blocks (causal bound c0 = 128*floor(j/2), j = kb-8tt):
      ST[kb] = KT_blk^T @ QT_tile   [128 k, 512-c0 q] (PSUM)
      PT = exp(scale * ST)          (ACT, PSUM->SBUF f16)
      strip blocks: PT[:, c0:c0+128] *= mask8[j]   (DVE f16; mask is
        tri / zeros / ones depending on parity -- per-core data)
      acc (+)= PT                   (DVE f16 accumulate; replaces the
                                     ones^T @ PT matmuls of the baseline)
      OT += V_blk^T @ PT            [128 h, 512 q] (PSUM accum)
    per i-block: lc = acc_blk^T @ ones -> 1/l (DVE reciprocal)
    O^T -> PE transpose -> per-partition scalar multiply -> store

Scheduling: the attention inner loop is paced by the scalar engine's exp
(529 ns/block vs the PE's 430 ns), so the NEXT round's projection matmuls
are fed to the PE in small chunks between attention steps (the "stream");
the PE then never idles waiting on exp. Round-0 x loads are split in half
so the first projections start as soon as half the data has landed.
"""

import numpy as np

B, T, E, H = 4, 4096, 1024, 128
P = 128
NB_E = E // P           # 8 contraction chunks
TQ = T // 2             # 2048 gathered queries per core
N_QT = TQ // 512        # 4 q-tiles per core
SCALE = float(H) ** -0.5
N_CORES = 8
F32 = np.float32
N_WARMUP = 0


def _query_rows(p: int) -> np.ndarray:
    """Absolute row indices of the gathered queries for piece p (in order)."""
    blocks = [np.arange(256 * g + 128 * p, 256 * g + 128 * p + 128) for g in range(16)]
    return np.concatenate(blocks)


def _mask8(p: int) -> np.ndarray:
    """mask8[kk, j, r]: multiplier for strip block j at query block i=c0/128.

    Query col 128*i + r is abs 1024*tt + 256*i + 128*p + r; key kk of block
    kb is abs 128*kb + kk. With j = kb - 8*tt, m = 2*i + p - j the block is
    fully visible (m>=1), triangular (m==0, visible iff kk<=r), or hidden
    (m<=-1). At i = floor(j/2): m = 2*floor(j/2) + p - j.
    """
    kk = np.arange(P)[:, None]
    r = np.arange(P)[None, :]
    tri = (kk <= r)
    m8 = np.empty((P, 8, P), dtype=np.float16)
    for j in range(8):
        m = 2 * (j // 2) + p - j
        if m >= 1:
            m8[:, j, :] = 1.0
        elif m == 0:
            m8[:, j, :] = tri.astype(np.float16)
        else:
            m8[:, j, :] = 0.0
    return m8


def _emit(tc, aps):
    from concourse import mybir
    from concourse.masks import make_identity

    nc = tc.nc
    f32 = mybir.dt.float32
    f16 = mybir.dt.float16
    EXP = mybir.ActivationFunctionType.Exp

    xT, xqT, wq, wk, wv, mask8, out = aps

    from contextlib import ExitStack

    ctx = ExitStack()
    with ctx:
        # ---- pools ----
        consts = ctx.enter_context(tc.tile_pool(name="consts", bufs=1))
        x_pool = ctx.enter_context(tc.tile_pool(name="x", bufs=14))
        vt_pool = ctx.enter_context(tc.tile_pool(name="vt", bufs=3))
        pt_pool = ctx.enter_context(tc.tile_pool(name="pt", bufs=8))
        acc_pool = ctx.enter_context(tc.tile_pool(name="acc", bufs=4))
        osb_pool = ctx.enter_context(tc.tile_pool(name="osb", bufs=2))
        on_pool = ctx.enter_context(tc.tile_pool(name="on", bufs=2))
        sm_pool = ctx.enter_context(tc.tile_pool(name="sm", bufs=8))
        s_ps = ctx.enter_context(tc.tile_pool(name="sps", bufs=3, space="PSUM"))
        proj_ps = ctx.enter_context(tc.tile_pool(name="pps", bufs=2, space="PSUM"))
        o_ps = ctx.enter_context(tc.tile_pool(name="ops", bufs=1, space="PSUM"))
        t_ps = ctx.enter_context(tc.tile_pool(name="tps", bufs=2, space="PSUM"))

        # ---- persistent SBUF tensors ----
        identity = consts.tile([P, P], f16)
        ones = consts.tile([P, 1], f16)
        wq_sb = consts.tile([P, NB_E, P], f16)
        wk_sb = consts.tile([P, NB_E, P], f16)
        wv_sb = consts.tile([P, NB_E, P], f16)
        mask_sb = consts.tile([P, 8, P], f16)
        kt_all = consts.tile([P, T], f16)
        v_all = consts.tile([P, T // P, P], f16)
        qt_all = consts.tile([P, TQ], f16)

        make_identity(nc, identity[:])
        nc.gpsimd.memset(ones[:], 1.0)

        # PE warmup: ramp the tensor-engine p-state during the initial DMA
        # window (transposes depend only on the on-device identity tile).
        for w in range(N_WARMUP):
            wp = t_ps.tile([P, P], f16, tag="tps", name=f"warm_{w}")
            nc.tensor.transpose(wp[:], identity[:], identity[:])

        def load_x(src_ap, t0):
            xt = x_pool.tile([P, NB_E, 512], f16, tag="x")
            nc.sync.dma_start(
                xt[:], src_ap[:, t0:t0 + 512].rearrange("(c p) t -> p c t", p=P))
            return xt

        def load_x_split(src_ap, t0):
            # two half-chunk DMAs in separate tiles: the first projection
            # matmuls can start after only half the data has landed
            halves = []
            for h in range(2):
                xt = x_pool.tile([P, NB_E // 2, 512], f16, tag="x")
                nc.sync.dma_start(
                    xt[:],
                    src_ap[h * 512:(h + 1) * 512, t0:t0 + 512]
                    .rearrange("(c p) t -> p c t", p=P))
                halves.append(xt)
            return halves

        # interleave weight loads with round-0 x loads so each projection's
        # dependencies arrive as early as possible
        nc.sync.dma_start(wq_sb[:], wq.rearrange("(c p) h -> p c h", p=P))
        xq_pre = load_x_split(xqT, 0)
        nc.sync.dma_start(wk_sb[:], wk.rearrange("(c p) h -> p c h", p=P))
        xk_pre = [load_x_split(xT, 0)]
        nc.sync.dma_start(wv_sb[:], wv.rearrange("(c p) h -> p c h", p=P))
        nc.sync.dma_start(mask_sb[:], mask8.rearrange("(j p) r -> p j r", p=P))
        xk_pre.append(load_x_split(xT, 512))

        def x_src(xt, c):
            return xt[c // 4][:, c % 4, :] if isinstance(xt, list) else xt[:, c, :]

        def project(w_sb, xt, dst_ap):
            ps = proj_ps.tile([P, 512], f32, tag="pps")
            for c in range(NB_E):
                nc.tensor.matmul(
                    ps[:],
                    lhsT=w_sb[:, c, :],
                    rhs=x_src(xt, c),
                    start=(c == 0),
                    stop=(c == NB_E - 1),
                )
            nc.vector.tensor_copy(dst_ap, ps[:])

        def v_transpose(vt, tok, u):
            kb = tok * 4 + u
            tp = t_ps.tile([P, P], f16, tag="tps", name=f"vt_{kb}")
            nc.tensor.transpose(tp[:], vt[:, u * P:(u + 1) * P], identity[:])
            nc.vector.tensor_copy(v_all[:, kb, :], tp[:])

        def do_kv(tok):
            xk = xk_pre[tok] if tok < 2 else load_x(xT, tok * 512)
            project(wk_sb, xk, kt_all[:, tok * 512:(tok + 1) * 512])
            vt = vt_pool.tile([P, 512], f16, tag="vt")
            project(wv_sb, xk, vt[:])
            for u in range(4):
                v_transpose(vt, tok, u)

        def do_q(tt):
            xq = xq_pre if tt == 0 else load_x(xqT, tt * 512)
            project(wq_sb, xq, qt_all[:, tt * 512:(tt + 1) * 512])

        # ---- projection step-stream: next-round projection work emitted in
        # small chunks between attention steps, so the tensor engine has
        # filler while it waits on exp results (the scalar engine paces the
        # attention inner loop otherwise) ----
        stream = []
        _psn = [0]

        def stream_project(w_sb, xt, dst_ap):
            ref = {}

            def mk(c):
                def f():
                    if c == 0:
                        _psn[0] += 1
                        ref['ps'] = proj_ps.tile(
                            [P, 512], f32, tag="pps", name=f"pps_{_psn[0]}")
                    nc.tensor.matmul(
                        ref['ps'][:],
                        lhsT=w_sb[:, c, :],
                        rhs=x_src(xt, c),
                        start=(c == 0),
                        stop=(c == NB_E - 1),
                    )
                return f

            for c in range(NB_E):
                stream.append(mk(c))
            stream.append(lambda: nc.vector.tensor_copy(dst_ap, ref['ps'][:]))

        def stream_kv(tok, xk=None):
            if xk is None:
                xk = xk_pre[tok] if tok < 2 else load_x(xT, tok * 512)
            stream_project(wk_sb, xk, kt_all[:, tok * 512:(tok + 1) * 512])
            vt = vt_pool.tile([P, 512], f16, tag="vt")
            stream_project(wv_sb, xk, vt[:])
            for u in range(4):
                stream.append(
                    (lambda uu: lambda: v_transpose(vt, tok, uu))(u))

        def stream_q(tt, xq=None):
            if xq is None:
                xq = load_x(xqT, tt * 512)
            stream_project(wq_sb, xq, qt_all[:, tt * 512:(tt + 1) * 512])

        def emit_stream(n):
            for _ in range(min(n, len(stream))):
                stream.pop(0)()

        for tt in range(N_QT):
            if tt == 0:
                do_q(0)
                do_kv(0)
            stream_kv(2 * tt + 1)
            # issue next round's x DMAs now (the matmul steps are scheduled
            # between the attention halves) so the loads land well ahead
            if tt + 1 < N_QT:
                xq_next = load_x(xqT, (tt + 1) * 512)
                xk_next = load_x(xT, (2 * tt + 2) * 512)
            else:
                xq_next = xk_next = None

            # ---- attention for q-tile tt ----
            qs = qt_all[:, tt * 512:(tt + 1) * 512]
            ot = o_ps.tile([P, 512], f32, tag="ops")
            acc = acc_pool.tile([P, 512], f16, tag="acc")
            o_sb = osb_pool.tile([P, 512], f16, tag="osb")
            on = on_pool.tile([P, 4, P], f32, tag="on")
            nkb = 8 * tt + 8

            def c0_of(kb):
                j = kb - 8 * tt
                return 128 * (j // 2) if j > 0 else 0

            s_tiles = {}

            def emit_scores(kb):
                c0 = c0_of(kb)
                s = s_ps.tile([P, 512], f32, tag="sps", name=f"s_{tt}_{kb}")
                nc.tensor.matmul(
                    s[:, c0:512],
                    lhsT=kt_all[:, kb * P:(kb + 1) * P],
                    rhs=qs[:, c0:512],
                    start=True,
                    stop=True,
                )
                s_tiles[kb] = s

            rlcs = [None] * 4

            def lc_block(i):
                # acc columns for block i are final once every kb with
                # c0 <= 128*i has run (kb = 8tt+2i+1); compute 1/l for the
                # block early so only the PSUM reads remain for the tail.
                lc = t_ps.tile([P, 1], f32, tag="tps", name=f"lc_{tt}_{i}")
                nc.tensor.matmul(
                    lc[:],
                    lhsT=acc[:, i * P:(i + 1) * P],
                    rhs=ones[:],
                    start=True,
                    stop=True,
                )
                rlc = sm_pool.tile([P, 1], f32, tag="rlc")
                nc.vector.reciprocal(rlc[:], lc[:])
                rlcs[i] = rlc

            pending_adds = []

            def mk_add(kb, c0, pt):
                def f():
                    if kb == 0:
                        nc.vector.tensor_copy(acc[:, c0:512], pt[:, c0:512])
                    else:
                        nc.vector.tensor_add(
                            acc[:, c0:512], acc[:, c0:512], pt[:, c0:512])
                return f

            def attn_half(kbs):
                pace = -(-len(stream) // len(kbs)) if kbs else 0
                for idx, kb in enumerate(kbs):
                    if idx == 0:
                        emit_scores(kb)
                        if len(kbs) > 1:
                            emit_scores(kbs[1])
                    if idx + 2 < len(kbs):
                        emit_scores(kbs[idx + 2])
                    s = s_tiles.pop(kb)
                    c0 = c0_of(kb)
                    j = kb - 8 * tt
                    pt = pt_pool.tile([P, 512], f16, tag="pt")
                    nc.scalar.activation(
                        pt[:, c0:512], s[:, c0:512], EXP, scale=SCALE)
                    if j >= 0:
                        nc.vector.tensor_mul(
                            pt[:, c0:c0 + P], pt[:, c0:c0 + P], mask_sb[:, j, :])
                    # defer the l-accumulate by 2 blocks so the mask multiply
                    # (which gates PV) isn't queued behind the add backlog
                    pending_adds.append(mk_add(kb, c0, pt))
                    if len(pending_adds) > 2:
                        pending_adds.pop(0)()
                    nc.tensor.matmul(
                        ot[:, c0:512],
                        lhsT=v_all[:, kb, :],
                        rhs=pt[:, c0:512],
                        start=(kb == 0),
                        stop=(kb == nkb - 1),
                    )
                    emit_stream(pace)

            attn_half(list(range(8 * tt + 4)))
            emit_stream(len(stream))    # kv(2tt+1) must be done for half B
            if tt + 1 < N_QT:
                stream_q(tt + 1, xq_next)
                stream_kv(2 * tt + 2, xk_next)
            attn_half(list(range(8 * tt + 4, nkb)))
            for f in pending_adds:      # acc must be complete before lc
                f()
            pending_adds.clear()
            emit_stream(len(stream))    # flush before round tt+1

            # ---- epilogue: normalize + transpose + store per column block ----
            for i in range(4):
                lc_block(i)
            last = tt == N_QT - 1
            nc.vector.tensor_copy(o_sb[:], ot[:])
            for i in range(4):
                tp = t_ps.tile([P, P], f16, tag="tps", name=f"otp_{tt}_{i}")
                nc.tensor.transpose(tp[:], o_sb[:, i * P:(i + 1) * P], identity[:])
                nc.vector.tensor_scalar_mul(on[:, i, :], tp[:], rlcs[i][:])
                if not last:
                    nc.sync.dma_start(
                        out[tt * 512 + i * P:tt * 512 + (i + 1) * P, :],
                        on[:, i, :])
            if last:
                # single store for the final tile: one queue dispatch instead
                # of four on the tail critical path
                nc.sync.dma_start(
                    out[tt * 512:(tt + 1) * 512, :].rearrange(
                        "(i p) h -> p i h", p=P),
                    on[:],
                )


def build_program():
    import concourse.tile as tile
    from concourse import bacc, mybir

    f32 = mybir.dt.float32
    f16 = mybir.dt.float16
    nc = bacc.Bacc("TRN2", target_bir_lowering=False, debug=False,
                   num_devices=N_CORES)
    xT = nc.dram_tensor("xT", [E, T], f16, kind="ExternalInput").ap()
    xqT = nc.dram_tensor("xqT", [E, TQ], f16, kind="ExternalInput").ap()
    wq = nc.dram_tensor("wq", [E, H], f16, kind="ExternalInput").ap()
    wk = nc.dram_tensor("wk", [E, H], f16, kind="ExternalInput").ap()
    wv = nc.dram_tensor("wv", [E, H], f16, kind="ExternalInput").ap()
    mask8 = nc.dram_tensor("mask8", [8 * P, P], f16, kind="ExternalInput").ap()
    out = nc.dram_tensor("out", [TQ, H], f32, kind="ExternalOutput").ap()

    with tile.TileContext(nc) as tc:
        _emit(tc, (xT, xqT, wq, wk, wv, mask8, out))
    nc.compile()
    return nc


def make_in_maps(x, Wq, Wk, Wv):
    """Per-core input maps. x: [B,T,E] f32; W*: [H,E] f32."""
    x = np.asarray(x, dtype=F32)
    wq_t = np.ascontiguousarray(np.asarray(Wq, dtype=F32).T.astype(np.float16))
    wk_t = np.ascontiguousarray(np.asarray(Wk, dtype=F32).T.astype(np.float16))
    wv_t = np.ascontiguousarray(np.asarray(Wv, dtype=F32).T.astype(np.float16))
    in_maps = []
    for c in range(N_CORES):
        b, p = c // 2, c % 2
        xb = x[b]                                              # [T, E]
        xT_np = np.ascontiguousarray(xb.T.astype(np.float16))
        xqT_np = np.ascontiguousarray(xb[_query_rows(p)].T.astype(np.float16))
        in_maps.append({
            "xT": xT_np,
            "xqT": xqT_np,
            "wq": wq_t,
            "wk": wk_t,
            "wv": wv_t,
            "mask8": _mask8(p).transpose(1, 0, 2).reshape(8 * P, P),
        })
    return in_maps


LDW_OPT = False


def _enable_ldw_opt():
    """Flip walrus's --enable-ldw-opt (defaults off in this toolchain): it
    elides redundant LDWEIGHTS when consecutive matmuls share a stationary
    operand. Correctness is covered by the output check."""
    import concourse.bass_utils as bu
    if getattr(bu, "_ldw_patched", False):
        return
    orig = bu.run_command

    def patched(cmd, *a, **kw):
        cmd = ["--enable-ldw-opt=true" if c == "--enable-ldw-opt=false" else c
               for c in cmd]
        return orig(cmd, *a, **kw)

    bu.run_command = patched
    bu._ldw_patched = True


def run(x, Wq, Wk, Wv, trace=False, trace_cores=None):
    """Returns (full_output [B,T,H] f32, BassKernelResults)."""
    from concourse.bass_utils import run_bass_kernel_spmd

    if LDW_OPT:
        _enable_ldw_opt()
    nc = build_program()
    in_maps = make_in_maps(x, Wq, Wk, Wv)
    res = run_bass_kernel_spmd(
        nc, in_maps, list(range(N_CORES)), trace=trace,
        trace_cores=trace_cores,
    )
    full = np.empty((B, T, H), dtype=F32)
    for c in range(N_CORES):
        b, p = c // 2, c % 2
        full[b, _query_rows(p), :] = res.results[c]["out"]
    return full, res


def kernel(x, Wq, Wk, Wv):
    full, _ = run(x, Wq, Wk, Wv, trace=False)
    return full


if __name__ == "__main__":
    nc = build_program()
    print("program built ok")
